# revision 1
# baseline (speedup 1.0000x reference)
"""Brute-force k=1 L2 nearest-neighbor retrieval on 8 trn2 NeuronCores.

Strategy: shard `keys` row-wise across the 8 cores (62500 rows each). Each
core streams its shard from HBM once and computes d_i = ||k_i - q||^2
(same argmin as the reference's ||k||^2 - 2 k.q, shifted by the constant
||q||^2), keeping a [128, 489] distance matrix in SBUF (partition p,
column c <-> shard row c*128 + p). A short on-device epilogue finds each
partition's min + column index (negate + Max8/MaxIndex8). The host reduces
the 8*128 candidates, maps the winner back to a global row, and returns
values[row] @ query.

Per-core engine budget per 1024-row supertile ([128, 8, 256] f32, 1 MiB):
DMA ~2.8us, DVE (tensor_sub + 1 group square-accum) ~2.7us, ACT (7 group
square-accums) ~2.8us -- balanced at the ~360 GB/s/core HBM roofline.
"""

import numpy as np

N, D = 500000, 256
CORES = 8
NS = N // CORES            # 62500 rows per core
P = 128
G = 8                      # row-groups per supertile
SUPER = P * G              # 1024 rows per supertile
NSUPER = NS // SUPER       # 61 full supertiles
TAIL = NS - NSUPER * SUPER  # 36 leftover rows
NCOL = NSUPER * G + 1      # 489 distance columns (tail in the last one)
PAD_VAL = 1.0e4            # pad rows -> distance ~2.6e10, never the min

_CACHE = {}


def _build():
    from concourse import bacc, tile
    from concourse import mybir

    nc = bacc.Bacc("TRN2", target_bir_lowering=False, debug=False,
                   num_devices=CORES)
    keys_in = nc.declare_dram_parameter("keys", [NS, D], mybir.dt.float32,
                                        isOutput=False)
    q_in = nc.declare_dram_parameter("q", [D], mybir.dt.float32,
                                     isOutput=False)
    nmax_out = nc.declare_dram_parameter("nmax", [P, 8], mybir.dt.float32,
                                         isOutput=True)
    nidx_out = nc.declare_dram_parameter("nidx", [P, 8], mybir.dt.uint32,
                                         isOutput=True)

    SQ = mybir.ActivationFunctionType.Square

    with tile.TileContext(nc) as tc:
        with tc.tile_pool(name="persist", bufs=1) as persist, \
             tc.tile_pool(name="ktiles", bufs=4) as ktiles, \
             tc.tile_pool(name="diffs", bufs=3) as diffs, \
             tc.tile_pool(name="scratch", bufs=2) as scratch:
            qt = persist.tile([P, D], mybir.dt.float32)
            nc.sync.dma_start(out=qt[:],
                              in_=q_in[:].unsqueeze(0).partition_broadcast(P))
            dist = persist.tile([P, NCOL], mybir.dt.float32)

            for tt in range(NSUPER):
                kt = ktiles.tile([P, G, D], mybir.dt.float32, tag="kt")
                nc.sync.dma_start(
                    out=kt[:],
                    in_=keys_in[tt * SUPER:(tt + 1) * SUPER]
                        .rearrange("(g p) d -> p g d", p=P))
                diff = diffs.tile([P, G, D], mybir.dt.float32, tag="diff")
                qb = qt[:].unsqueeze(1).broadcast_to([P, G, D])
                nc.vector.tensor_sub(diff[:], kt[:], qb)
                for g in range(G):
                    col = tt * G + g
                    if g == G - 1:
                        # one group per supertile on DVE to unload ACT
                        sq_v = scratch.tile([P, D], mybir.dt.float32,
                                            tag="sq_v")
                        nc.vector.scalar_tensor_tensor(
                            out=sq_v[:], in0=diff[:, g, :], scalar=0.0,
                            in1=diff[:, g, :],
                            op0=mybir.AluOpType.bypass,
                            op1=mybir.AluOpType.mult,
                            accum_out=dist[:, col:col + 1])
                    else:
                        sq_a = scratch.tile([P, D], mybir.dt.float32,
                                            tag="sq_a")
                        nc.scalar.activation(
                            sq_a[:], diff[:, g, :], SQ,
                            accum_out=dist[:, col:col + 1])

            # ragged tail: 36 real rows, 92 partitions of PAD_VAL filler
            ktail = ktiles.tile([P, D], mybir.dt.float32, tag="ktail")
            nc.vector.memset(ktail[:], PAD_VAL)
            nc.sync.dma_start(out=ktail[:TAIL, :],
                              in_=keys_in[NSUPER * SUPER:])
            dtail = diffs.tile([P, D], mybir.dt.float32, tag="dtail")
            nc.vector.tensor_sub(dtail[:], ktail[:], qt[:])
            sq_t = scratch.tile([P, D], mybir.dt.float32, tag="sq_a")
            nc.scalar.activation(sq_t[:], dtail[:], SQ,
                                 accum_out=dist[:, NCOL - 1:NCOL])

            # per-partition argmin: negate then Max8 + MaxIndex8
            neg = persist.tile([P, NCOL], mybir.dt.float32)
            nc.vector.tensor_scalar_mul(neg[:], dist[:], -1.0)
            nmax = persist.tile([P, 8], mybir.dt.float32)
            nidx = persist.tile([P, 8], mybir.dt.uint32)
            nc.vector.max(nmax[:], neg[:])
            nc.vector.max_index(nidx[:], nmax[:], neg[:])
            nc.sync.dma_start(out=nmax_out[:], in_=nmax[:])
            nc.sync.dma_start(out=nidx_out[:], in_=nidx[:])

    nc.compile()
    return nc


def get_nc():
    if "nc" not in _CACHE:
        _CACHE["nc"] = _build()
    return _CACHE["nc"]


def kernel(query, keys, values):
    from concourse.bass_utils import run_bass_kernel_spmd

    query = np.asarray(query, dtype=np.float32)
    keys = np.asarray(keys, dtype=np.float32)
    values = np.asarray(values, dtype=np.float32)
    assert keys.shape == (N, D) and query.shape == (D,)

    nc = get_nc()
    in_maps = [
        {"keys": keys[c * NS:(c + 1) * NS], "q": query} for c in range(CORES)
    ]
    res = run_bass_kernel_spmd(nc, in_maps, core_ids=list(range(CORES)))

    # reduce the 8*128 (value, column) candidates on the host
    vals = np.stack([-res.results[c]["nmax"][:, 0] for c in range(CORES)])
    cols = np.stack([res.results[c]["nidx"][:, 0] for c in range(CORES)])
    c, p = np.unravel_index(np.argmin(vals), vals.shape)
    row = c * NS + int(cols[c, p]) * P + p
    assert row < N
    out = values[row].astype(np.float32) @ query
    return np.asarray([out], dtype=np.float32)
